# revision 1
# baseline (speedup 1.0000x reference)
"""Trainium2 Bass kernel for expected-calibration-error (ECE) over [N,C] logits.

Contract: kernel(logits, targets) -> np.float32 scalar (shape ()), matching

    probs = softmax(logits); conf = max(probs); pred = argmax(probs)
    acc = (pred == targets); bins of width 1/10 over (k/10, (k+1)/10]
    ECE = sum_k |avg_conf_k - avg_acc_k| * count_k / N

Strategy (data-parallel over 8 NeuronCores, rows sharded):
  * Host packs the class index into the low 7 mantissa bits of each logit
    (value perturbation <= 1.5e-5 relative). A single segmented reduce_max
    per row then yields BOTH the max logit and, in its low mantissa bits,
    the argmax class — one DVE pass instead of two.
  * conf = exp(max) / sum_c exp(logit_c)  (no max-subtraction needed:
    |logits| < ~6 so exp() is safe in f32).  exp runs on the scalar
    (activation) engine; the 128-wide row sums are folded 128->64 on
    GpSimd, then segment-reduced on the vector engine.
  * Per-tile heavy passes only write per-row (max, sumexp) columns into
    group buffers; all the small per-row ops (conf, acc, bin masks) are
    batched once per GROUP of tiles so tiny cross-engine-dependent ops
    don't head-of-line-block the in-order engines.
  * Per-row bin membership is encoded as cumulative masks
    g[k] = (conf > k/10), k = 0..10, and the per-bin (count, sum_conf,
    sum_acc) triples are produced by the tensor engine: for each block of
    128 rows, matmul(lhsT=[ones|conf|acc] (128x3), rhs=g (128x11))
    accumulates [3,11] cumulative stats in PSUM across the whole shard.
  * Host sums the 8 [3,11] outputs, differences adjacent cumulative
    columns to get per-bin stats, and applies the final ECE formula.
"""

import numpy as np

# Problem constants (hardcoded per harness contract).
N = 262144
C = 128
P = 128          # SBUF partitions
NB = 10          # calibration bins
NCORES = 8
T = 32           # rows per partition per tile
GK = 4           # tiles per small-op group
ROWS_PER_CORE = N // NCORES          # 32768
NTILES = ROWS_PER_CORE // (P * T)    # 8

_CACHE = {}

# perf-tuning knobs used by kernel()
KNOBS = dict(gp_fold=True, gp_maxfold_tiles=0, act_accum_blocks=0)


def build(ntiles=NTILES, t_rows=T, gk=GK, gp_fold=True, gp_maxfold_tiles=0,
          act_accum_blocks=0):
    """Build the Bass module. Returns nc.

    gp_fold: fold the exp() row halves 128->64 on GpSimd before the DVE
        row-sum reduce (halves DVE reduce_sum work).
    gp_maxfold_tiles: for the first k tiles of every group, also fold the
        row max 128->64 on GpSimd before the DVE reduce_max (DVE/GpSimd
        load-balance knob).
    act_accum_blocks: for the first k 128-row blocks of every tile, compute
        the row sum-of-exp on the scalar engine via per-block activation
        accum_out (skips the GpSimd fold and DVE reduce for those blocks).
    """
    import concourse.bacc as bacc
    import concourse.tile as tile
    from concourse import mybir

    f32 = mybir.dt.float32
    i32 = mybir.dt.int32
    Alu = mybir.AluOpType
    Act = mybir.ActivationFunctionType
    X = mybir.AxisListType.X

    assert ntiles % gk == 0
    ngroups = ntiles // gk
    gw = gk * t_rows  # group width (row-blocks per group)
    ab = act_accum_blocks
    assert 0 <= ab <= t_rows
    rb = t_rows - ab  # blocks on the fold+reduce path

    nc = bacc.Bacc(trn_type="TRN2")

    y_d = nc.dram_tensor("y", [ntiles, P, t_rows * C], f32, kind="ExternalInput")
    # tcode grouped to allow one DMA per group: [ngroups, P, gw]
    tcode_d = nc.dram_tensor("tcode", [ngroups, P, gw], i32, kind="ExternalInput")
    thr_d = nc.dram_tensor("thr", [1, NB + 1], f32, kind="ExternalInput")
    out_d = nc.dram_tensor("gstats", [3, NB + 1], f32, kind="ExternalOutput")

    with tile.TileContext(nc) as tc:
        with (
            tc.tile_pool(name="io", bufs=4) as io_pool,
            tc.tile_pool(name="ep", bufs=2) as e_pool,
            tc.tile_pool(name="fp", bufs=2) as f_pool,
            tc.tile_pool(name="grp", bufs=2) as grp_pool,
            tc.tile_pool(name="single", bufs=1) as single,
            tc.tile_pool(name="psum", bufs=1, space="PSUM") as psum_pool,
        ):
            thr_sb = single.tile([P, NB + 1], f32)
            nc.sync.dma_start(out=thr_sb[:], in_=thr_d[:].partition_broadcast(P))
            c127 = single.tile([P, 1], i32)
            nc.vector.memset(c127[:], 127)

            pstats = psum_pool.tile([3, NB + 1], f32)

            for grp in range(ngroups):
                # --- phase 1: heavy per-tile passes -> group stat columns ---
                my_g = grp_pool.tile([P, gw], f32)
                s_g = grp_pool.tile([P, gw], f32)
                tc_g = grp_pool.tile([P, gw], i32)
                nc.sync.dma_start(out=tc_g[:], in_=tcode_d[grp])

                for ti in range(gk):
                    t = grp * gk + ti
                    o0, o1 = ti * t_rows, (ti + 1) * t_rows

                    y_t = io_pool.tile([P, t_rows * C], f32)
                    nc.sync.dma_start(out=y_t[:], in_=y_d[t])
                    y3 = y_t[:].rearrange("p (t c) -> p t c", c=C)

                    # row sums of exp(y):
                    #  - first `ab` blocks: scalar-engine exp with accum_out
                    #  - rest: big exp, GpSimd 128->64 fold, DVE reduce
                    if ab > 0:
                        Es = f_pool.tile([P, C], f32, name="Escratch")
                        for b in range(ab):
                            nc.scalar.activation(
                                out=Es[:], in_=y3[:, b, :], func=Act.Exp,
                                accum_out=s_g[:, o0 + b : o0 + b + 1],
                            )
                    if rb > 0:
                        E = e_pool.tile([P, rb * C], f32)
                        nc.scalar.activation(
                            out=E[:], in_=y_t[:, ab * C :], func=Act.Exp
                        )
                        E3 = E[:].rearrange("p (t c) -> p t c", c=C)

                        if gp_fold:
                            F = f_pool.tile([P, rb * (C // 2)], f32)
                            F3 = F[:].rearrange("p (t c) -> p t c", c=C // 2)
                            nc.gpsimd.tensor_tensor(
                                out=F3, in0=E3[:, :, 0 : C // 2],
                                in1=E3[:, :, C // 2 : C], op=Alu.add,
                            )
                            nc.vector.tensor_reduce(
                                out=s_g[:, o0 + ab : o1], in_=F3, axis=X, op=Alu.add
                            )
                        else:
                            nc.vector.tensor_reduce(
                                out=s_g[:, o0 + ab : o1], in_=E3, axis=X, op=Alu.add
                            )

                    # packed row max (value + argmax in low mantissa bits)
                    if ti < gp_maxfold_tiles:
                        M = f_pool.tile([P, t_rows * (C // 2)], f32, name="Mfold")
                        M3 = M[:].rearrange("p (t c) -> p t c", c=C // 2)
                        nc.gpsimd.tensor_tensor(
                            out=M3, in0=y3[:, :, 0 : C // 2],
                            in1=y3[:, :, C // 2 : C], op=Alu.max,
                        )
                        nc.vector.tensor_reduce(
                            out=my_g[:, o0:o1], in_=M3, axis=X, op=Alu.max
                        )
                    else:
                        nc.vector.tensor_reduce(
                            out=my_g[:, o0:o1], in_=y3, axis=X, op=Alu.max
                        )

                # --- phase 2: batched small ops over the whole group ---
                maxE = grp_pool.tile([P, gw], f32)
                nc.scalar.activation(out=maxE[:], in_=my_g[:], func=Act.Exp)
                rs = grp_pool.tile([P, gw], f32)
                nc.vector.reciprocal(out=rs[:], in_=s_g[:])

                rhs3 = grp_pool.tile([P, 3, gw], f32)
                nc.gpsimd.memset(rhs3[:, 0, :], 1.0)
                nc.vector.tensor_tensor(
                    out=rhs3[:, 1, :], in0=maxE[:], in1=rs[:], op=Alu.mult
                )

                # acc: (packed-max mantissa & 127) == (127 - target)
                accx = grp_pool.tile([P, gw], i32)
                nc.vector.scalar_tensor_tensor(
                    out=accx[:], in0=my_g[:].bitcast(i32), scalar=c127[:],
                    in1=tc_g[:], op0=Alu.bitwise_and, op1=Alu.bitwise_xor,
                )
                nc.vector.tensor_scalar(
                    out=rhs3[:, 2, :], in0=accx[:], scalar1=0, scalar2=None,
                    op0=Alu.is_equal,
                )

                # cumulative bin masks g[k] = conf > k/10
                g = grp_pool.tile([P, gw, NB + 1], f32)
                conf_b = rhs3[:, 1, :].unsqueeze(2).broadcast_to([P, gw, NB + 1])
                thr_b = thr_sb[:].unsqueeze(1).broadcast_to([P, gw, NB + 1])
                nc.vector.tensor_tensor(out=g[:], in0=conf_b, in1=thr_b, op=Alu.is_gt)

                # per-128-row-block cumulative histogram triples on PE
                for j in range(gw):
                    nc.tensor.matmul(
                        pstats[:],
                        rhs3[:, :, j],
                        g[:, j, :],
                        start=(grp == 0 and j == 0),
                        stop=(grp == ngroups - 1 and j == gw - 1),
                        skip_group_check=True,
                    )

            stats_sb = single.tile([3, NB + 1], f32)
            nc.vector.tensor_copy(out=stats_sb[:], in_=pstats[:])
            nc.sync.dma_start(out=out_d[:], in_=stats_sb[:])

    nc.compile()
    return nc


def prep_inputs(logits, targets, ntiles=NTILES, t_rows=T, gk=GK, ncores=NCORES):
    """Pack + shard host inputs. Returns list of per-core in_maps."""
    l = np.ascontiguousarray(np.asarray(logits, dtype=np.float32))
    tg = np.asarray(targets).astype(np.int64)
    n = l.shape[0]

    yb = l.view(np.int32) & np.int32(~127)
    yb = yb | (127 - np.arange(C, dtype=np.int32))[None, :]
    y = yb.view(np.float32)

    tcode = (127 - tg).astype(np.int32)
    thr = (np.arange(NB + 1, dtype=np.float32) / NB).reshape(1, NB + 1)

    ngroups = ntiles // gk
    rpc = n // ncores
    in_maps = []
    for k in range(ncores):
        yk = y[k * rpc : (k + 1) * rpc].reshape(ntiles, P, t_rows * C)
        tk = (
            tcode[k * rpc : (k + 1) * rpc]
            .reshape(ngroups, gk, P, t_rows)
            .transpose(0, 2, 1, 3)
            .reshape(ngroups, P, gk * t_rows)
        )
        in_maps.append({"y": yk, "tcode": np.ascontiguousarray(tk), "thr": thr})
    return in_maps


def finalize(gstats_list, n=N):
    """Combine per-core cumulative [3, 11] stats into the ECE scalar."""
    G = np.zeros((3, NB + 1), dtype=np.float64)
    for gs in gstats_list:
        G += gs.astype(np.float64)
    per = G[:, 0:NB] - G[:, 1 : NB + 1]
    counts, sum_conf, sum_acc = per[0], per[1], per[2]
    safe = np.maximum(counts, 1.0)
    avg_conf = sum_conf / safe
    avg_acc = sum_acc / safe
    prop = counts / float(n)
    ece = np.where(counts > 0, np.abs(avg_conf - avg_acc) * prop, 0.0).sum()
    return np.array(ece, dtype=np.float32)


LAST_RESULTS = None  # BassKernelResults of the most recent kernel() call


def kernel(logits, targets):
    global LAST_RESULTS
    from concourse.bass_utils import run_bass_kernel_spmd

    key = (NTILES, T, GK, tuple(sorted(KNOBS.items())))
    if key not in _CACHE:
        _CACHE[key] = build(NTILES, T, GK, **KNOBS)
    nc = _CACHE[key]

    in_maps = prep_inputs(logits, targets)
    res = run_bass_kernel_spmd(nc, in_maps, core_ids=list(range(NCORES)))
    LAST_RESULTS = res
    return finalize([r["gstats"] for r in res.results])



# revision 10
# speedup vs baseline: 1.1070x; 1.1070x over previous
"""Trainium2 Bass kernel for expected-calibration-error (ECE) over [N,C] logits.

Contract: kernel(logits, targets) -> np.float32 scalar (shape ()), matching

    probs = softmax(logits); conf = max(probs); pred = argmax(probs)
    acc = (pred == targets); bins of width 1/10 over (k/10, (k+1)/10]
    ECE = sum_k |avg_conf_k - avg_acc_k| * count_k / N

Strategy (data-parallel over 8 NeuronCores, rows sharded):
  * Host converts logits to fp16 and gathers the target-class logit per
    row (tl). This halves HBM traffic (the memory roofline) and unlocks
    the DVE 2x perf mode (2-byte dtypes) for both row reductions. ECE is
    an aggregate over 262k rows with a 2e-2 tolerance; fp16's 1e-3-level
    noise on conf and ~0.2%-of-rows argmax-tie noise on acc are far below
    the gate.
  * Per tile [128 partitions, 32 rows, 128 classes]:
      - scalar engine: E = exp(y16) in fp16 (one big activation)
      - vector engine: row max via fp16 tensor_reduce (2x mode)
      - row sumexp: gpsimd folds E halves (add) and the vector engine
        reduces the folded half (fp16 2x) -- balances DVE vs GpSimd.
  * acc = (tl == rowmax), exact in the fp16 domain (tl is one of the
    row's values, so equality holds iff the target attains the max).
  * conf = exp(rowmax) * reciprocal(sumexp).
  * Bin masks as SIGN masks on the scalar engine (Exp and Sign share an
    activation table set): g'[k] = sign(conf - k/10) in fp16. The tensor
    engine accumulates cumulative [3,11] (count,conf,acc) sign-stats in
    PSUM via one tiny fp16 matmul per 128-row block.
  * Host recovers true cumulative stats G = (G' + G'[:,0:1])/2, sums the
    8 cores, differences adjacent columns, applies the ECE formula.
"""

import numpy as np

# Problem constants (hardcoded per harness contract).
N = 262144
C = 128
P = 128          # SBUF partitions
NB = 10          # calibration bins
NCORES = 8
T = 32           # rows per partition per tile
GK = 2           # tiles per small-op group
ROWS_PER_CORE = N // NCORES          # 32768
NTILES = ROWS_PER_CORE // (P * T)    # 8

_CACHE = {}

# perf-tuning knobs used by kernel()
# sum_fold_skip: tile indices whose sumexp skips the gpsimd fold and uses a
#   direct DVE reduce instead (DVE/GpSimd load-balance).
KNOBS = dict(sum_fold_skip=(0,))


def build(ntiles=NTILES, t_rows=T, gk=GK, sum_fold_skip=(0,)):
    """Build the Bass module. Returns nc."""
    import concourse.bacc as bacc
    import concourse.tile as tile
    from concourse import mybir

    f32 = mybir.dt.float32
    f16 = mybir.dt.float16
    Alu = mybir.AluOpType
    Act = mybir.ActivationFunctionType
    X = mybir.AxisListType.X

    assert ntiles % gk == 0
    ngroups = ntiles // gk
    gw = gk * t_rows  # group width (row-blocks per group)
    tw = ntiles * t_rows  # total row-blocks per core

    nc = bacc.Bacc(trn_type="TRN2")

    y_d = nc.dram_tensor("y", [ntiles, P, t_rows * C], f16, kind="ExternalInput")
    tl_d = nc.dram_tensor("tl", [P, tw], f16, kind="ExternalInput")
    thr_d = nc.dram_tensor("thr", [1, NB + 1], f32, kind="ExternalInput")
    out_d = nc.dram_tensor("gstats", [3, NB + 1], f32, kind="ExternalOutput")

    with tile.TileContext(nc) as tc:
        with (
            tc.tile_pool(name="io", bufs=4) as io_pool,
            tc.tile_pool(name="ep", bufs=3) as e_pool,
            tc.tile_pool(name="fp", bufs=2) as f_pool,
            tc.tile_pool(name="grp", bufs=3) as grp_pool,
            tc.tile_pool(name="single", bufs=1) as single,
            tc.tile_pool(name="psum", bufs=1, space="PSUM") as psum_pool,
        ):
            # Issue the first y-tile DMAs as early as possible.
            y_tiles = {}
            for t in range(min(2, ntiles)):
                y_t = io_pool.tile([P, t_rows * C], f16, name="y_t")
                nc.sync.dma_start(out=y_t[:], in_=y_d[t])
                y_tiles[t] = y_t

            tl_all = single.tile([P, tw], f16)
            nc.sync.dma_start(out=tl_all[:], in_=tl_d[:])
            thr_sb = single.tile([P, NB + 1], f32)
            nc.sync.dma_start(out=thr_sb[:], in_=thr_d[:].partition_broadcast(P))

            pstats = psum_pool.tile([3, NB + 1], f32)

            def phase1(grp):
                my_g = grp_pool.tile([P, gw], f16, name="my_g")
                s_g = grp_pool.tile([P, gw], f16, name="s_g")
                for ti in range(gk):
                    t = grp * gk + ti
                    o0, o1 = ti * t_rows, (ti + 1) * t_rows

                    y_t = y_tiles.pop(t, None)
                    if y_t is None:
                        y_t = io_pool.tile([P, t_rows * C], f16, name="y_t")
                        nc.sync.dma_start(out=y_t[:], in_=y_d[t])
                    y3 = y_t[:].rearrange("p (t c) -> p t c", c=C)

                    # row max via fp16 DVE reduce (2x perf mode)
                    nc.vector.tensor_reduce(
                        out=my_g[:, o0:o1], in_=y3, axis=X, op=Alu.max
                    )

                    # row sums of exp(y): fp16 exp on the scalar engine
                    E = e_pool.tile([P, t_rows * C], f16)
                    nc.scalar.activation(out=E[:], in_=y_t[:], func=Act.Exp)
                    E3 = E[:].rearrange("p (t c) -> p t c", c=C)
                    with nc.allow_low_precision("fp16 sumexp; ECE tol 2e-2"):
                        if t in sum_fold_skip:
                            nc.vector.tensor_reduce(
                                out=s_g[:, o0:o1], in_=E3, axis=X, op=Alu.add
                            )
                        else:
                            M = f_pool.tile([P, t_rows * (C // 2)], f16)
                            M3 = M[:].rearrange("p (t c) -> p t c", c=C // 2)
                            nc.gpsimd.tensor_tensor(
                                out=M3, in0=E3[:, :, 0 : C // 2],
                                in1=E3[:, :, C // 2 : C], op=Alu.add,
                            )
                            nc.vector.tensor_reduce(
                                out=s_g[:, o0:o1], in_=M3, axis=X, op=Alu.add
                            )
                return my_g, s_g

            def phase2(grp, my_g, s_g):
                maxE = grp_pool.tile([P, gw], f32)
                nc.scalar.activation(out=maxE[:], in_=my_g[:], func=Act.Exp)
                s32 = grp_pool.tile([P, gw], f32)
                nc.vector.tensor_copy(out=s32[:], in_=s_g[:])
                rs = grp_pool.tile([P, gw], f32)
                nc.vector.reciprocal_approx_fast(out=rs[:], in_=s32[:])

                rhs3 = grp_pool.tile([P, 3, gw], f16)
                nc.gpsimd.memset(rhs3[:, 0, :], 1.0)
                conf = grp_pool.tile([P, gw], f32)
                nc.vector.tensor_tensor(
                    out=conf[:], in0=maxE[:], in1=rs[:], op=Alu.mult
                )
                nc.vector.tensor_copy(out=rhs3[:, 1, :], in_=conf[:])

                # acc: target logit attains the row max (exact in fp16)
                nc.vector.tensor_tensor(
                    out=rhs3[:, 2, :], in0=my_g[:],
                    in1=tl_all[:, grp * gw : (grp + 1) * gw], op=Alu.is_equal,
                )

                # cumulative bin SIGN masks g'[k] = sign(conf - k/10) on the
                # scalar engine (same act table set as Exp, so no reload).
                g = grp_pool.tile([P, NB + 1, gw], f16)
                for k in range(NB + 1):
                    nc.scalar.activation(
                        out=g[:, k, :], in_=conf[:], func=Act.Sign,
                        bias=thr_sb[:, k : k + 1],
                    )

                # per-128-row-block cumulative histogram triples on PE
                for j in range(gw):
                    nc.tensor.matmul(
                        pstats[:],
                        rhs3[:, :, j],
                        g[:, :, j],
                        start=(grp == 0 and j == 0),
                        stop=(grp == ngroups - 1 and j == gw - 1),
                        skip_group_check=True,
                    )

            prev = None
            for grp in range(ngroups):
                cur = phase1(grp)
                if prev is not None:
                    phase2(grp - 1, *prev)
                prev = cur
            phase2(ngroups - 1, *prev)

            stats_sb = single.tile([3, NB + 1], f32)
            nc.vector.tensor_copy(out=stats_sb[:], in_=pstats[:])
            nc.sync.dma_start(out=out_d[:], in_=stats_sb[:])

    nc.compile()
    return nc


def prep_inputs(logits, targets, ntiles=NTILES, t_rows=T, ncores=NCORES):
    """Convert + shard host inputs. Returns list of per-core in_maps."""
    l = np.asarray(logits, dtype=np.float32)
    tg = np.asarray(targets).astype(np.int64)
    n = l.shape[0]

    y16 = l.astype(np.float16)
    tl16 = y16[np.arange(n), tg]
    # negated thresholds, used as per-partition bias for the Sign masks
    thr = -(np.arange(NB + 1, dtype=np.float32) / NB).reshape(1, NB + 1)

    rpc = n // ncores
    tw = ntiles * t_rows
    in_maps = []
    for k in range(ncores):
        yk = y16[k * rpc : (k + 1) * rpc].reshape(ntiles, P, t_rows * C)
        tlk = (
            tl16[k * rpc : (k + 1) * rpc]
            .reshape(ntiles, P, t_rows)
            .transpose(1, 0, 2)
            .reshape(P, tw)
        )
        in_maps.append(
            {"y": np.ascontiguousarray(yk), "tl": np.ascontiguousarray(tlk),
             "thr": thr}
        )
    return in_maps


def finalize(gstats_list, n=N):
    """Combine per-core cumulative sign-stats [3, 11] into the ECE scalar."""
    Gp = np.zeros((3, NB + 1), dtype=np.float64)
    for gs in gstats_list:
        Gp += gs.astype(np.float64)
    # sign masks: G'[j,k] = 2*G[j,k] - S_j with S_j = G'[j,0]
    G = (Gp + Gp[:, 0:1]) / 2.0
    per = G[:, 0:NB] - G[:, 1 : NB + 1]
    counts, sum_conf, sum_acc = per[0], per[1], per[2]
    safe = np.maximum(counts, 1.0)
    avg_conf = sum_conf / safe
    avg_acc = sum_acc / safe
    prop = counts / float(n)
    ece = np.where(counts > 0, np.abs(avg_conf - avg_acc) * prop, 0.0).sum()
    return np.array(ece, dtype=np.float32)


LAST_RESULTS = None  # BassKernelResults of the most recent kernel() call


def kernel(logits, targets):
    global LAST_RESULTS
    from concourse.bass_utils import run_bass_kernel_spmd

    key = (NTILES, T, GK, tuple(sorted(KNOBS.items())))
    if key not in _CACHE:
        _CACHE[key] = build(NTILES, T, GK, **KNOBS)
    nc = _CACHE[key]

    in_maps = prep_inputs(logits, targets)
    res = run_bass_kernel_spmd(nc, in_maps, core_ids=list(range(NCORES)))
    LAST_RESULTS = res
    return finalize([r["gstats"] for r in res.results])
